# revision 36
# baseline (speedup 1.0000x reference)
"""DeepSeek-V2 normal MoE layer on 8 Trainium2 NeuronCores.

Expert-parallel sharding: core e holds expert e's weights (cast to bf16).
The router (tiny [T,E] matmul + softmax + top-k) runs on the host in fp32 —
this is the dispatch step of the sharding layer: it decides which token rows
are copied to which core. Each core receives its routed tokens (bf16,
host-packed so every DMA moves contiguous partition rows) plus a broadcast
row of the renormalized top-k combine weights. On device, each core computes
the gated-SiLU expert MLP for its tokens (three 2048/1408-contraction matmul
phases in bf16 with fp32 PSUM accumulation, feature-major layout so no
on-device transposes are needed), applies the combine weight in the fp32
output epilogue, and the host scatter-adds the per-expert outputs back into
the full [T, H] result.

Measured DMA model: per-queue throughput is ~88-150 GB/s (sync/scalar
hardware queues ~90, gpsimd software queue ~140), aggregate ~300 GB/s,
with ~0.6 us trigger + ~1 us first-transfer cost per DMA — so transfers
are few and wide, spread across all three trigger engines:
  - wg+wu h-interleaved pairs per i-tile [P, HT, 2, P]; i-tile 0 streams
    as three pieces (sync, sync, gpsimd) sized so arrival tracks the
    h-loop; later i-tiles stream as two halves on sync || gpsimd
  - x in eight 2-h pieces alternating scalar/gpsimd in consumption order
  - wd quad-packed [P, 4*IT*P] on gpsimd (idle during phase 2 otherwise)
  - y outputs in bf16 (halves the flush; ~1e-3 of the 2e-2 error budget),
    one block per ht alternating sync/scalar, the last block split into
    two half-partition DMAs so the kernel tail flushes in parallel
The first i-tile is DMA-supply-paced (x 2.27 MiB + weights 1 MiB against
~300 GB/s), so the PE warm-up uses full-width 128x512 matmuls on zeros:
the clock governor's boost->dip->sustain sequence then plays out inside
the DMA-wait window and real matmuls start at the sustained full clock.
PSUM uses one 8-bank pool for both phases (pg0/pu0/pg1/pu1, all bufs=2);
phase 2's accumulators ride phase 1's tags so the phase transition
resolves through normal buffer rotation with no drain stall.

Per-core capacity C = max tokens routed to any expert (rounded up to 2);
pad token columns are zero and carry combine-weight 0, so their
contribution is exactly zero.
"""

import numpy as np
import ml_dtypes


def _ensure_ntff_hook():
    """This image's antenv package lacks axon_hooks, but concourse's
    run_bass_kernel_spmd unconditionally imports it when BASS_TRACE is set.
    Provide the module (and the ctypes NTFF hook from trn_agent_boot, when
    available) so tracing works instead of crashing. Idempotent; never
    overwrites an existing module."""
    import sys
    import types
    try:
        import antenv  # noqa: F401
    except ImportError:
        return
    if "antenv.axon_hooks" in sys.modules:
        return
    try:
        import antenv.axon_hooks  # noqa: F401
        return
    except ImportError:
        pass
    mod = types.ModuleType("antenv.axon_hooks")
    holder = {"h": None}
    mod.set_axon_ntff_profile_hook = lambda h: holder.__setitem__("h", h)
    mod.get_axon_ntff_profile_hook = lambda: holder.get("h")
    sys.modules["antenv.axon_hooks"] = mod
    import antenv as _a
    _a.axon_hooks = mod
    try:
        from trn_agent_boot.trn_boot import _ntff_profile_via_ctypes
        hook = _ntff_profile_via_ctypes("/opt/axon/libaxon_pjrt.so")
        if hook is not None:
            mod.set_axon_ntff_profile_hook(hook)
    except Exception:
        pass


_ensure_ntff_hook()

H = 2048
I_DIM = 1408
E = 8
P = 128
HT = H // P      # 16
IT = I_DIM // P  # 11

_compiled = {}
last_results = None


def _chunks(C):
    """Token-column chunks of <=512 (one PSUM bank / max moving free dim)."""
    out = []
    s = 0
    while s < C:
        w = min(512, C - s)
        out.append((s, w))
        s += w
    return out


def _build(C):
    import concourse.bacc as bacc
    import concourse.mybir as mybir
    import concourse.tile as tile

    dt = mybir.dt
    nc = bacc.Bacc("TRN2", target_bir_lowering=False)
    # Host-packed layouts (see module docstring). Base tiling: block
    # [t, p, k*128+c] = W[k*128+p, t*128+c] of the natural layout, i.e.
    # partition p of block t holds that block's full contraction row.
    xg = nc.dram_tensor("xg", [P, HT * C], dt.bfloat16, kind="ExternalInput")
    wt = nc.dram_tensor("wt", [P, C], dt.float32, kind="ExternalInput")
    # gate/up pairs, h-interleaved per i-tile: [IT, P, HT, 2, P] — any
    # h-range is a contiguous per-partition run, so each i-tile can stream
    # as two half DMAs on two queues in parallel (per-queue ~88-150 GB/s).
    wgu = nc.dram_tensor("wgu", [IT, P, HT * 2 * P], dt.bfloat16,
                         kind="ExternalInput")
    # down-proj quads: [HT//4, P, 4, IT, P]
    wdq = nc.dram_tensor("wdq", [HT // 4, P, 4 * IT * P], dt.bfloat16,
                         kind="ExternalInput")
    # y returns in bf16: halves output-DMA bytes (the tail flush is on the
    # kernel's critical path); adds ~1e-3 of the 2e-2 error budget.
    yt = nc.dram_tensor("yt", [H, C], dt.bfloat16, kind="ExternalOutput")

    ch = _chunks(C)
    W2 = C - 512 if C > 512 else 0

    with tile.TileContext(nc) as tc:
        with (
            tc.tile_pool(name="xpool", bufs=1) as xpool,
            tc.tile_pool(name="apool", bufs=1) as apool,
            tc.tile_pool(name="wpool", bufs=5) as wpool,
            tc.tile_pool(name="wdpool", bufs=3) as wdpool,
            tc.tile_pool(name="spool", bufs=2) as spool,
            tc.tile_pool(name="ypool", bufs=3) as ypool,
        ):
            # PE warm-up while head DMAs stream: the clock governor's
            # boost->dip->sustain sequence triggers on HEAVY load onset, so
            # the warm-up must look like real work (full 128x512 matmuls on
            # zeros) — the half-clock dip then plays out inside the DMA-wait
            # window instead of on top of the first real i-tiles.
            warm = spool.tile([P, 512], dt.bfloat16, name="warm", tag="warm")
            nc.gpsimd.memset(warm[:], 0.0)

            # Head schedule (it=0 is supply-paced: it needs x 2.27 MiB +
            # wgu0 1 MiB while aggregate DMA is ~300 GB/s, so pieces are
            # interleaved across the three queues in h-consumption order):
            #   sync   (~88 GB/s): wgu0 h0-3, wgu0 h4-9, then it>=1 h0-7
            #   scalar (~95 GB/s): x h0-1, h4-5, h8-9, h12-13, wb
            #   gpsimd (~140GB/s): x h2-3, h6-7, wgu0 h10-15, x h10-11,
            #                      h14-15, then it>=1 h8-15, then wd quads
            wparts0 = []  # it=0 weight pieces: (tile, h_start, h_count)

            def wgu_piece(pool, it, hs, hn, eng, tag):
                t = pool.tile([P, hn, 2, P], dt.bfloat16, name=tag, tag=tag)
                eng.dma_start(out=t[:],
                              in_=wgu[it, :, hs * 2 * P:(hs + hn) * 2 * P])
                return (t, hs, hn)

            xparts = []  # (tile, h_start, h_count)

            def xpiece(eng, hs, hn):
                t = xpool.tile([P, hn * C], dt.bfloat16, name=f"x{hs}",
                               tag=f"x{hs}")
                eng.dma_start(out=t[:], in_=xg[:, hs * C:(hs + hn) * C])
                xparts.append((t, hs, hn))

            wparts0.append(wgu_piece(xpool, 0, 0, 4, nc.sync, "w0a"))
            wparts0.append(wgu_piece(xpool, 0, 4, 6, nc.sync, "w0b"))
            xpiece(nc.scalar, 0, 2)
            xpiece(nc.gpsimd, 2, 2)
            xpiece(nc.scalar, 4, 2)
            xpiece(nc.gpsimd, 6, 2)
            xpiece(nc.scalar, 8, 2)
            xpiece(nc.gpsimd, 10, 2)
            wparts0.append(wgu_piece(xpool, 0, 10, 3, nc.gpsimd, "w0c"))
            wparts0.append(wgu_piece(xpool, 0, 13, 3, nc.gpsimd, "w0d"))
            xpiece(nc.scalar, 12, 2)
            xpiece(nc.gpsimd, 14, 2)
            wb = xpool.tile([P, C], dt.float32, name="wb", tag="wb")
            nc.scalar.dma_start(out=wb[:], in_=wt[:, :])

            def part_h(parts, h):
                for t, hs, hn in parts:
                    if hs <= h < hs + hn:
                        return t, h - hs
                raise AssertionError(h)

            def xg_h(h):
                t, hh = part_h(xparts, h)
                return t[:, hh * C:(hh + 1) * C]

            # One PSUM pool for both phases, 8 banks, everything
            # double-buffered: pg0 2 + pu0 2 + pg1 2 + pu1 2. Phase 2's
            # accumulators ride phase 1's tags, so their buffer rotation
            # naturally resolves the phase transition without a PSUM-drain
            # stall and without extra banks.
            with (
                tc.tile_pool(name="pp1", bufs=2, space="PSUM") as pp1,
            ):
                # Phase 1: A[i, t] = silu(G) * U, feature-major, per i-tile.
                a_t = []
                for it in range(IT):
                    if it == 0:
                        wparts = wparts0
                    else:
                        # halves stream on two queues in parallel
                        wparts = [
                            wgu_piece(wpool, it, 0, HT // 2, nc.sync, "wa"),
                            wgu_piece(wpool, it, HT // 2, HT // 2,
                                      nc.gpsimd, "wbt"),
                        ]
                    pg0 = pp1.tile([P, ch[0][1]], dt.float32, name="pg0",
                                   tag="pg0", bufs=2)
                    pu0 = pp1.tile([P, ch[0][1]], dt.float32, name="pu0",
                                   tag="pu0", bufs=2)
                    if W2:
                        pg1 = pp1.tile([P, W2], dt.float32, name="pg1",
                                       tag="pg1", bufs=2)
                        pu1 = pp1.tile([P, W2], dt.float32, name="pu1",
                                       tag="pu1", bufs=2)
                        pgs = [pg0, pg1]
                        pus = [pu0, pu1]
                    else:
                        pgs, pus = [pg0], [pu0]
                    if it == 0:
                        for _ in range(14):
                            nc.tensor.matmul(pg0[:], warm[:, :P],
                                             warm[:, :512], start=True, stop=True)
                    for h in range(HT):
                        st, sp = h == 0, h == HT - 1
                        wt_t, hh = part_h(wparts, h)
                        wg_h = wt_t[:, hh, 0, :]
                        wu_h = wt_t[:, hh, 1, :]
                        xh = xg_h(h)
                        for ci, (s, w) in enumerate(ch):
                            nc.tensor.matmul(pgs[ci][:], wg_h,
                                             xh[:, s:s + w], start=st, stop=sp)
                        for ci, (s, w) in enumerate(ch):
                            nc.tensor.matmul(pus[ci][:], wu_h,
                                             xh[:, s:s + w], start=st, stop=sp)
                    sg = spool.tile([P, C], dt.float32, name="sg", tag="sg")
                    ai = apool.tile([P, C], dt.bfloat16, name=f"a{it}", tag=f"a{it}")
                    for ci, (s, w) in enumerate(ch):
                        nc.scalar.activation(sg[:, s:s + w], pgs[ci][:],
                                             mybir.ActivationFunctionType.Silu)
                        nc.vector.tensor_mul(ai[:, s:s + w], sg[:, s:s + w], pus[ci][:])
                    a_t.append(ai)

                # Phase 2: Y^T[h, t] = sum_i Wd[i, h] * A[i, t].
                for htq in range(HT // 4):
                    wdt = wdpool.tile([P, 4, IT, P], dt.bfloat16,
                                      name="wd", tag="wd")
                    nc.gpsimd.dma_start(out=wdt[:], in_=wdq[htq, :, :])
                    for hq in range(4):
                        ht = htq * 4 + hq
                        py0 = pp1.tile([P, ch[0][1]], dt.float32, name="py0",
                                       tag="pg0", bufs=2)
                        pys = [py0]
                        if W2:
                            py1 = pp1.tile([P, W2], dt.float32, name="py1",
                                           tag="pg1", bufs=2)
                            pys.append(py1)
                        for i2 in range(IT):
                            st, sp = i2 == 0, i2 == IT - 1
                            for ci, (s, w) in enumerate(ch):
                                nc.tensor.matmul(pys[ci][:], wdt[:, hq, i2, :],
                                                 a_t[i2][:, s:s + w],
                                                 start=st, stop=sp)
                        yo = ypool.tile([P, C], dt.bfloat16, name="yo", tag="yo")
                        for ci, (s, w) in enumerate(ch):
                            nc.vector.tensor_mul(yo[:, s:s + w], wb[:, s:s + w],
                                                 pys[ci][:])
                        if ht == HT - 1:
                            # tail: two half-partition flushes on parallel
                            # queues instead of one serial block
                            nc.sync.dma_start(
                                out=yt[ht * P:ht * P + 64, :], in_=yo[:64, :])
                            nc.scalar.dma_start(
                                out=yt[ht * P + 64:(ht + 1) * P, :], in_=yo[64:, :])
                        else:
                            yeng = nc.sync if ht % 2 == 0 else nc.scalar
                            yeng.dma_start(out=yt[ht * P:(ht + 1) * P, :],
                                           in_=yo[:])
    nc.compile()
    return nc


def _tile_weight(w, nt_out):
    """[K, N] -> [N/128, 128, K] blocks: out[t, p, k*128+c] = w[k*128+p, t*128+c]."""
    K, N = w.shape
    kt = K // P
    return np.ascontiguousarray(
        w.reshape(kt, P, nt_out, P).transpose(2, 1, 0, 3).reshape(nt_out, P, kt * P)
    )


def _pack_expert(wg_e, wu_e, wd_e):
    """Pack one expert's weights into the wide-row DMA layouts."""
    bf16 = ml_dtypes.bfloat16
    gt = _tile_weight(wg_e.astype(bf16), IT)   # [IT, P, HT*P]
    ut = _tile_weight(wu_e.astype(bf16), IT)
    dtl = _tile_weight(wd_e.astype(bf16), HT)  # [HT, P, IT*P]
    # gate/up h-interleaved pairs: [IT, P, HT, 2, P]
    wgup = np.ascontiguousarray(
        np.stack([gt.reshape(IT, P, HT, P), ut.reshape(IT, P, HT, P)], axis=3)
        .reshape(IT, P, HT * 2 * P))
    # wd quads: [HT//4, P, 4*IT*P]
    wdqp = np.ascontiguousarray(
        dtl.reshape(HT // 4, 4, P, IT * P).transpose(0, 2, 1, 3)
        .reshape(HT // 4, P, 4 * IT * P))
    return wgup, wdqp


def kernel(hidden_states, gate_w, w_gate, w_up, w_down, top_k):
    global last_results
    hs = np.ascontiguousarray(np.asarray(hidden_states, dtype=np.float32))
    gw = np.asarray(gate_w, dtype=np.float32)
    wg_all = np.asarray(w_gate, dtype=np.float32)
    wu_all = np.asarray(w_up, dtype=np.float32)
    wd_all = np.asarray(w_down, dtype=np.float32)
    K = int(np.asarray(top_k))
    T = hs.shape[0]
    if K <= 0:
        return np.zeros((T, H), np.float32)

    # ---- router (mirrors the reference numerics in fp32) ----
    logits = hs @ gw.T
    m = logits.max(-1, keepdims=True)
    ex = np.exp(logits - m)
    probs = ex / ex.sum(-1, keepdims=True)
    order = np.argsort(-probs, axis=-1, kind="stable")
    topi = order[:, :K]
    topv = np.take_along_axis(probs, topi, axis=-1)
    topv = topv / topv.sum(-1, keepdims=True)

    # ---- dispatch: gather each expert's tokens ----
    idxs, wvs = [], []
    for e in range(E):
        mask = topi == e
        rows = np.nonzero(mask.any(-1))[0]
        idxs.append(rows)
        wvs.append(topv[mask].astype(np.float32))
    counts = [len(r) for r in idxs]
    C = max(64, ((max(counts) + 1) // 2) * 2)

    nc = _compiled.get(C)
    if nc is None:
        nc = _compiled[C] = _build(C)

    bf16 = ml_dtypes.bfloat16
    in_maps = []
    for e in range(E):
        idx, wv = idxs[e], wvs[e]
        n = len(idx)
        xsel = hs[idx]  # [n, H]
        xg_np = np.zeros((HT, P, C), dtype=bf16)
        xg_np[:, :, :n] = xsel.T.astype(bf16).reshape(HT, P, n)
        xg_np = np.ascontiguousarray(xg_np.transpose(1, 0, 2).reshape(P, HT * C))
        wt_np = np.zeros((P, C), dtype=np.float32)
        wt_np[:, :n] = wv[None, :]
        wgup, wdqp = _pack_expert(wg_all[e], wu_all[e], wd_all[e])
        in_maps.append({
            "xg": xg_np,
            "wt": wt_np,
            "wgu": wgup,
            "wdq": wdqp,
        })

    from concourse.bass_utils import run_bass_kernel_spmd
    res = run_bass_kernel_spmd(nc, in_maps, core_ids=list(range(E)))
    last_results = res

    # ---- combine: scatter-add per-expert outputs ----
    out = np.zeros((T, H), np.float32)
    for e in range(E):
        idx = idxs[e]
        n = len(idx)
        yt_e = res.results[e]["yt"]  # [H, C] bf16
        out[idx] += yt_e[:, :n].T.astype(np.float32)
    return out
